# revision 32
# baseline (speedup 1.0000x reference)
"""BiLSTM classifier on 8 TRN2 NeuronCores.

Sharding: batch 4-way x direction 2-way. Core r handles batch quarter
q = r//2 (16 sequences) and LSTM direction d = r%2 for BOTH layers.
Backward-direction cores see their input time-reversed so the device
program is identical on all cores (pure SPMD). Layer-0 hidden states are
exchanged between the (fwd, bwd) core pairs with per-chunk AllGathers so
each core can build the concat(fwd, bwd) input projection for layer 1.
Each core returns its 16 sequences' partial FC output (own direction's
half of the pooled features); host adds the pair and concatenates.

This revision minimizes the per-timestep recurrence LOOP LATENCY (the
wall-clock is 2T x loop-latency; batch splitting cannot reduce it):
 - ONE merged chain per step (all 16 sequences, N=16 matmuls).
 - Gates PSUM split (i,f,g | o): sigma(ifg) is on the critical path;
   sigma(o) runs in its shadow (only needed for the final h multiply).
 - The xp seed matmuls for step t+1 and the next chunk's input-projection
   GEMM pieces are emitted in the PE's idle window after each burst.
 - xp PSUM drains and the layer-1 pooled-sum accumulation run on the
   otherwise idle GPSIMD engine.
 - tanh(g) is reconstructed as 2*sigmoid(2g)-1 on the vector engine
   (g-gate weight rows pre-scaled by 2 on the host); bf16 everywhere
   except the c-state path.
"""
import sys

if '/opt/trn_rl_repo' not in sys.path:
    sys.path.insert(0, '/opt/trn_rl_repo')

import numpy as np

import concourse.bass as bass
import concourse.mybir as mybir
from concourse import tile
from concourse.bass_utils import run_bass_kernel_spmd
from concourse.vector_clock import ScopedClock

B, T, DIN, H, NCLS = 64, 1024, 12, 256, 17
NCORES = 8
BSH = B // (NCORES // 2)      # 16 sequences per core
CHUNK = 64
H4 = 4 * H                    # 1024
KH = H // 128                 # 2 k-tiles per hidden state
M4 = H4 // 128                # 8 m-tiles of gates
F32 = mybir.dt.float32
BF16 = mybir.dt.bfloat16
AF = mybir.ActivationFunctionType
ALU = mybir.AluOpType


def _patch_tile_drain():
    """Walrus in this env rejects >1 sync-wait on one instruction; spread the
    final Tile drain's waits across sync-engine nops."""
    def _drain_and_barrier(self, tick_clock, wait_clock):
        drain_inst = self.nc.sync.drain()
        wait_clock.add_sem_waits(
            drain_inst.ins, ScopedClock({None: tick_clock.global_clock}))
        si = drain_inst.ins.sync_info
        if si is not None and len(si.on_wait) > 1:
            waits = list(si.on_wait)
            drain_inst.ins.sync_info = mybir.SyncInfo(
                on_wait=[waits[0]], on_update=list(si.on_update))
            for w in waits[1:]:
                nop = self.nc.sync.nop(nofuse=True)
                nop.ins.sync_info = mybir.SyncInfo(on_wait=[w], on_update=[])
        self.nc.all_engine_barrier()
        assert self.sems is not None
        popped = self.nc._tile_sem_poison_stack.pop()
        assert popped is self._sem_poison
        self.nc.clear_and_free_semaphores(list(self.sems.allocated().values()))
        self.nc.all_engine_barrier()

    tile.TileContext._drain_and_barrier = _drain_and_barrier


_patch_tile_drain()


def _split_multi_waits(nc):
    """This env's walrus supports only one sync-wait per instruction: move
    extra waits onto same-engine nops inserted just before the instruction."""
    cnt = 0
    for fn in nc.m.functions:
        for bb in fn.blocks:
            new = []
            changed = False
            for inst in bb.instructions:
                si = inst.sync_info
                if si is not None and len(si.on_wait) > 1:
                    changed = True
                    waits = list(si.on_wait)
                    for w in waits[:-1]:
                        nop = mybir.InstNoOp(
                            name=f"waitsplit_{cnt}", ins=[], outs=[])
                        cnt += 1
                        nop.engine = inst.engine
                        nop.sync_info = mybir.SyncInfo(
                            on_wait=[w], on_update=[])
                        new.append(nop)
                    inst.sync_info = mybir.SyncInfo(
                        on_wait=[waits[-1]], on_update=list(si.on_update))
                new.append(inst)
            if changed:
                bb.instructions = new


def build_nc(t_len=T, use_collective=True):
    nch = t_len // CHUNK
    nc = bass.Bass(num_devices=NCORES)

    # ---- external parameters (per-core data, identical program) ----
    xT_ext = nc.declare_dram_parameter("xT", [DIN + 1, t_len * BSH], F32,
                                       isOutput=False)
    whh0_ext = nc.declare_dram_parameter("whh0T", [H, H4], F32, isOutput=False)
    wih0_ext = nc.declare_dram_parameter("wih0T", [DIN + 1, H4], F32,
                                         isOutput=False)
    whh1_ext = nc.declare_dram_parameter("whh1T", [H, H4], F32, isOutput=False)
    wih1_ext = nc.declare_dram_parameter("wih1Te", [3 * H, H4], F32,
                                         isOutput=False)
    b1_ext = nc.declare_dram_parameter("b1", [1, H4], F32, isOutput=False)
    ident_ext = nc.declare_dram_parameter("ident", [128, 128], F32,
                                          isOutput=False)
    fcw_ext = nc.declare_dram_parameter("fcwT", [H, NCLS], F32, isOutput=False)
    fcb_ext = nc.declare_dram_parameter("fcb", [BSH, NCLS], F32, isOutput=False)
    out_ext = nc.declare_dram_parameter("out", [BSH, NCLS], F32, isOutput=True)

    # ---- dram scratch for the pairwise h0 exchange ----
    excin = []
    excout = []
    for c in range(nch):
        excin.append(nc.dram_tensor(f"excin{c}", [128, CHUNK, KH, BSH], BF16))
        excout.append(
            nc.dram_tensor(f"excout{c}", [2, 128, CHUNK, KH, BSH], BF16))
    groups = [[2 * q, 2 * q + 1] for q in range(NCORES // 2)]

    with tile.TileContext(nc) as tc:
        with (
            tc.tile_pool(name="const", bufs=1) as constp,
            tc.tile_pool(name="stage", bufs=2) as stagep,
            tc.tile_pool(name="step", bufs=4) as stepp,
            tc.tile_pool(name="gp", bufs=1, space=bass.MemorySpace.PSUM) as gpp,
            tc.tile_pool(name="xps", bufs=1, space=bass.MemorySpace.PSUM) as xpsp,
        ):
            # ---------- load + cast weights ----------
            def load_bf16(ext, rows, cols, tag):
                if rows <= 128:
                    out_t = constp.tile([rows, cols], BF16, tag=tag)
                    st = stagep.tile([128, cols], F32, tag="wstage")
                    nc.sync.dma_start(st[:rows, :], ext[:, :])
                    nc.vector.tensor_copy(out_t[:], st[:rows, :])
                else:
                    out_t = constp.tile([128, rows // 128, cols], BF16, tag=tag)
                    for i in range(rows // 128):
                        st = stagep.tile([128, cols], F32, tag="wstage")
                        nc.sync.dma_start(st[:], ext[i * 128:(i + 1) * 128, :])
                        nc.vector.tensor_copy(out_t[:, i, :], st[:])
                return out_t

            FP8 = mybir.dt.float8e4

            def load_fp8(ext, rows, cols, tag):
                out_t = constp.tile([128, rows // 128, cols], FP8, tag=tag)
                for i in range(rows // 128):
                    st = stagep.tile([128, cols], F32, tag="wstage")
                    nc.sync.dma_start(st[:], ext[i * 128:(i + 1) * 128, :])
                    nc.vector.tensor_copy(out_t[:, i, :], st[:])
                return out_t

            whh0_sb = load_fp8(whh0_ext, H, H4, "whh0")   # [128, KH, H4]
            whh1_sb = load_fp8(whh1_ext, H, H4, "whh1")
            wih1_sb = load_bf16(wih1_ext, 3 * H, H4, "wih1")  # [128, 6, H4]
            wih0_sb = load_bf16(wih0_ext, DIN + 1, H4, "wih0")  # [13, H4]
            b1_sb = load_bf16(b1_ext, 1, H4, "b1")         # [1, H4]
            fcw_sb = load_bf16(fcw_ext, H, NCLS, "fcw")    # [128, KH, NCLS]

            ident_sb = load_bf16(ident_ext, 128, 128, "ident")
            fcb_sb = constp.tile([BSH, NCLS], F32)
            nc.sync.dma_start(fcb_sb[:], fcb_ext[:])
            ones_sb = constp.tile([1, 512], BF16)
            nc.gpsimd.memset(ones_sb[:], 1.0)

            xT_sb = constp.tile([DIN + 1, t_len * BSH], BF16)
            for j in range(t_len * BSH // 1024):
                st = stagep.tile([128, 1024], F32, tag="wstage")
                nc.sync.dma_start(st[:DIN + 1, :],
                                  xT_ext[:, j * 1024:(j + 1) * 1024])
                nc.vector.tensor_copy(xT_sb[:, j * 1024:(j + 1) * 1024],
                                      st[:DIN + 1, :])

            # ---------- persistent state ----------
            # h0 store: slot s = 1 + local step; slot 0 is the zero init.
            # Slot-major layout so each step's h write is contiguous (DVE 2x).
            h0_sb = constp.tile([128, t_len + 1, KH, BSH], BF16)
            nc.gpsimd.memset(h0_sb[:, 0, :, :], 0.0)
            h1r = constp.tile([128, 2, KH, BSH], BF16)   # layer-1 h ring
            nc.gpsimd.memset(h1r[:, 1, :, :], 0.0)
            c_state = constp.tile([128, KH, BSH], F32)
            h1sum = constp.tile([128, KH, BSH], F32)
            nc.gpsimd.memset(h1sum[:], 0.0)

            # xp buffers: [128, CHUNK, M4, BSH] so one step's slice is a
            # contiguous 128-column block (fast seed matmul rhs).
            xp_sb = [constp.tile([128, CHUNK, M4, BSH], BF16, tag=f"xp{j}",
                                 name=f"xp{j}") for j in range(2)]
            # partner h for layer-1 xp (both directions), ping-pong by chunk
            hg_sb = [[constp.tile([128, CHUNK, KH, BSH], BF16,
                                  tag=f"hg{j}{dd}", name=f"hg{j}{dd}")
                      for dd in range(2)] for j in range(2)]
            # gates psum, ping-pong by step parity. Gate order is (f,g,i,o)
            # host-side: sigma(f,g) fires after its own 8 matmuls and the
            # fc/tg vector work overlaps the i,o matmuls.
            gpfg = [gpp.tile([128, 4, BSH], F32, tag=f"gpfg{j}",
                             name=f"gpfg{j}") for j in range(2)]
            gpio = [gpp.tile([128, 4, BSH], F32, tag=f"gpio{j}",
                             name=f"gpio{j}") for j in range(2)]
            xps = [xpsp.tile([128, 512], F32, tag=f"xps{j}", name=f"xps{j}")
                   for j in range(3)]
            fcps = xpsp.tile([BSH, NCLS], F32, tag="fcps")

            piece_ctr = [0]

            def xp_pieces(layer, c):
                """Build the emission thunks computing xp for chunk c into
                xp_sb[c % 2]. Returns {'pe': [...], 'drain': [...], 'mpp': k}:
                PE matmul thunks go into the post-burst PE window; each
                piece's PSUM drain is a DVE copy placed in the idle window
                after the h multiply, lagging its matmuls by >= 2 pieces so
                its WAR semaphore can never stall the PE queue."""
                xp_t = xp_sb[c % 2]
                pe_thunks = []
                drain_thunks = []
                for m in range(M4):
                    for half in range(2):
                        tn = 512 // BSH          # 32 timesteps per piece
                        t0 = half * tn
                        p = piece_ctr[0]
                        piece_ctr[0] += 1
                        ps = xps[p % 3]

                        def mk_mm(m=m, half=half, t0=t0, ps=ps, kk=None):
                            def emit():
                                if layer == 0:
                                    nc.tensor.matmul(
                                        ps[:],
                                        wih0_sb[:, m * 128:(m + 1) * 128],
                                        xT_sb[:, (c * CHUNK + t0) * BSH:
                                              (c * CHUNK + t0) * BSH + 512],
                                        start=True, stop=True)
                                else:
                                    if kk < KH:
                                        rhs = h0_sb[:,
                                                    1 + c * CHUNK + t0:
                                                    1 + c * CHUNK + t0 + tn,
                                                    kk, :]
                                        lhs = wih1_sb[:, kk,
                                                      m * 128:(m + 1) * 128]
                                    elif kk < 3 * KH:
                                        g = (kk - KH) // KH
                                        k2 = (kk - KH) % KH
                                        rhs = hg_sb[c % 2][g][:, t0:t0 + tn,
                                                              k2, :]
                                        lhs = wih1_sb[:, kk,
                                                      m * 128:(m + 1) * 128]
                                    else:
                                        rhs = ones_sb[:1, :]
                                        lhs = b1_sb[:, m * 128:(m + 1) * 128]
                                    nc.tensor.matmul(
                                        ps[:], lhs, rhs,
                                        start=(kk == 0), stop=(kk == 6))
                            return emit

                        if layer == 0:
                            pe_thunks.append(mk_mm())
                        else:
                            for kk in range(7):
                                pe_thunks.append(mk_mm(kk=kk))

                        def mk_drain(m=m, t0=t0, tn=tn, ps=ps):
                            def emit():
                                nc.vector.tensor_copy(
                                    xp_t[:, t0:t0 + tn, m, :], ps[:])
                            return emit
                        drain_thunks.append(mk_drain())
                return {"pe": pe_thunks, "drain": drain_thunks,
                        "mpp": 1 if layer == 0 else 7,
                        "pe_done": 0, "dr_done": 0}

            def emit_rest(sh):
                """Emit any remaining xp thunks, preserving the invariant
                that drain(p) is emitted before any matmul of piece p+3
                (which reuses its PSUM buffer)."""
                mpp = sh["mpp"]
                while sh["dr_done"] < len(sh["drain"]):
                    cap = (sh["dr_done"] + 3) * mpp
                    if sh["pe_done"] < min(cap, len(sh["pe"])):
                        t = min(sh["pe_done"] + mpp, cap, len(sh["pe"]))
                        while sh["pe_done"] < t:
                            sh["pe"][sh["pe_done"]]()
                            sh["pe_done"] += 1
                    else:
                        sh["drain"][sh["dr_done"]]()
                        sh["dr_done"] += 1

            def hg_load(c):
                """Load both directions' layer-0 h for chunk c (layer-1 xp)."""
                src = excout[nch - 1 - c]
                for dd in range(2):
                    nc.sync.dma_start(hg_sb[c % 2][dd][:],
                                      src[dd][:, ::-1, :, :])

            def seed_fg(tau):
                """Seed the f,g gates psum for step tau with xp."""
                par = tau % 2
                xp_t = xp_sb[(tau // CHUNK) % 2]
                nc.tensor.matmul(gpfg[par][:], ident_sb[:],
                                 xp_t[:, tau % CHUNK, 0:4, :],
                                 start=True, stop=False)

            def seed_io(tau):
                """Seed the i,o gates psum for step tau with xp."""
                par = tau % 2
                xp_t = xp_sb[(tau // CHUNK) % 2]
                nc.tensor.matmul(gpio[par][:], ident_sb[:],
                                 xp_t[:, tau % CHUNK, 4:8, :],
                                 start=True, stop=False)

            def lstm_step(layer, tau, whh_sb, shadow):
                """One timestep, all 16 sequences. `shadow` is a deque-like
                list of thunks for next-chunk xp work, emitted into the PE /
                Pool idle windows."""
                par = tau % 2
                # ---- recurrent matmuls: f,g tiles first, i,o tiles last;
                # the io seed rides inside the burst (off the pre-burst path)
                for m in range(M4):
                    if m == 4:
                        seed_io(tau)
                    for k in range(KH):
                        if layer == 0:
                            rhs = h0_sb[:, tau, k, :]
                        else:
                            rhs = h1r[:, (tau + 1) % 2, k, :]
                        nc.tensor.matmul(
                            gpfg[par][:, m, :] if m < 4 else
                            gpio[par][:, m - 4, :],
                            whh_sb[:, k, m * 128:(m + 1) * 128],
                            rhs, start=False,
                            stop=(m in (3, 7) and k == KH - 1))
                # ---- activations: sigma(f,g) first, then sigma(i,o) ----
                sfg = stepp.tile([128, 4, BSH], BF16, tag="sfg")
                nc.scalar.activation(sfg[:], gpfg[par][:], AF.Sigmoid,
                                     scale=1.0 / 16.0)
                sio = stepp.tile([128, 4, BSH], BF16, tag="sio")
                nc.scalar.activation(sio[:], gpio[par][:], AF.Sigmoid,
                                     scale=1.0 / 16.0)
                # ---- PE shadow work: next-chunk xp pieces ----
                if shadow is not None:
                    nmm = 3 if layer else 1
                    cap = min((shadow["dr_done"] + 3) * shadow["mpp"],
                              len(shadow["pe"]))
                    for _ in range(nmm):
                        if shadow["pe_done"] < cap:
                            shadow["pe"][shadow["pe_done"]]()
                            shadow["pe_done"] += 1
                # ---- DVE chain: fc, tg, ig, c ----
                tg_t = stepp.tile([128, KH, BSH], BF16, tag="tg", name="tg")
                nc.vector.tensor_scalar(tg_t[:], sfg[:, 2:4, :], 2.0, -1.0,
                                        ALU.mult, ALU.add)
                fc_t = stepp.tile([128, KH, BSH], F32, tag="fc", name="fc")
                nc.vector.tensor_mul(fc_t[:], sfg[:, 0:2, :], c_state[:])
                ig_t = stepp.tile([128, KH, BSH], BF16, tag="ig", name="ig")
                nc.vector.tensor_mul(ig_t[:], sio[:, 0:2, :], tg_t[:])
                nc.vector.tensor_add(c_state[:], fc_t[:], ig_t[:])
                # ---- tanh(c) and h ----
                tc_t = stepp.tile([128, KH, BSH], BF16, tag="tc", name="tc")
                nc.scalar.activation(tc_t[:], c_state[:], AF.Tanh)
                # warm-up: tiny junk matmul gated on tc wakes the PE ~200ns
                # before the next burst so it doesn't start in a cold p-state
                if tau + 1 < t_len:
                    nc.tensor.matmul(fcps[0:1, 0:1], tc_t[:, 0, 0:1],
                                     ident_sb[:, 0:1],
                                     start=True, stop=True,
                                     skip_group_check=True)
                    # fg seed runs warm right behind the dummy, just
                    # before the next burst (io seeds inside the burst)
                    seed_fg(tau + 1)
                if layer == 0:
                    hdst = h0_sb[:, tau + 1, :, :]
                else:
                    hdst = h1r[:, tau % 2, :, :]
                nc.vector.tensor_mul(hdst, sio[:, 2:4, :], tc_t[:])
                if layer == 1:
                    nc.gpsimd.tensor_add(h1sum[:], h1sum[:], hdst)
                # ---- DVE idle window: one xp PSUM drain, lagging its MMs ----
                if shadow is not None and shadow["dr_done"] < len(
                        shadow["drain"]):
                    pieces_ready = shadow["pe_done"] // shadow["mpp"]
                    if (shadow["dr_done"] < pieces_ready - 1
                            or shadow["pe_done"] >= len(shadow["pe"])):
                        shadow["drain"][shadow["dr_done"]]()
                        shadow["dr_done"] += 1

            # ================= layer 0 =================
            nc.gpsimd.memset(c_state[:], 0.0)
            emit_rest(xp_pieces(0, 0))   # prologue: chunk 0 xp
            seed_fg(0)
            for c in range(nch):
                shadow = xp_pieces(0, c + 1) if c + 1 < nch else None
                for tl in range(CHUNK):
                    lstm_step(0, c * CHUNK + tl, whh0_sb, shadow)
                if shadow is not None:
                    emit_rest(shadow)
                # stage + exchange this chunk
                nc.sync.dma_start(
                    excin[c][:],
                    h0_sb[:, 1 + c * CHUNK:1 + (c + 1) * CHUNK, :, :])
                if use_collective:
                    nc.gpsimd.collective_compute(
                        "AllGather", ALU.bypass,
                        replica_groups=groups,
                        ins=[excin[c][:]], outs=[excout[c][:]])
                else:
                    nc.sync.dma_start(excout[c][0], excin[c][:])
                    nc.sync.dma_start(excout[c][1], excin[c][:])

            # ================= layer 1 =================
            nc.gpsimd.memset(c_state[:], 0.0)
            hg_load(0)
            emit_rest(xp_pieces(1, 0))   # prologue: chunk 0 xp
            seed_fg(0)
            for c in range(nch):
                if c + 1 < nch:
                    hg_load(c + 1)
                    shadow = xp_pieces(1, c + 1)
                else:
                    shadow = None
                for tl in range(CHUNK):
                    lstm_step(1, c * CHUNK + tl, whh1_sb, shadow)
                if shadow is not None:
                    emit_rest(shadow)

            # ---------- pool + FC ----------
            pooled = stepp.tile([128, KH, BSH], BF16, tag="pooled")
            nc.scalar.activation(pooled[:], h1sum[:], AF.Identity,
                                 scale=1.0 / t_len)
            for k in range(KH):
                nc.tensor.matmul(fcps[:], pooled[:, k, :], fcw_sb[:, k, :],
                                 start=(k == 0), stop=(k == KH - 1))
            out_sb = stepp.tile([BSH, NCLS], F32, tag="outsb")
            nc.vector.tensor_add(out_sb[:], fcps[:], fcb_sb[:])
            nc.sync.dma_start(out_ext[:], out_sb[:])

    _split_multi_waits(nc)
    return nc


GPERM = np.concatenate([np.arange(H, 2 * H), np.arange(2 * H, 3 * H),
                        np.arange(0, H), np.arange(3 * H, 4 * H)])


def make_in_maps(x, w_ih0, w_hh0, b_ih0, b_hh0, w_ih1, w_hh1, b_ih1, b_hh1,
                 fc_w, fc_b, t_len=T):
    f32 = np.float32
    gsl = slice(2 * H, 3 * H)       # g-gate rows along the 4H axis
    in_maps = []
    for r in range(NCORES):
        q, d = r // 2, r % 2
        xs = np.asarray(x[BSH * q:BSH * q + BSH, :t_len], dtype=f32)
        if d == 1:
            xs = xs[:, ::-1, :]
        xT = np.concatenate([
            xs.transpose(2, 1, 0).reshape(DIN, t_len * BSH),
            np.ones((1, t_len * BSH), dtype=f32)], axis=0)

        whh0T = np.asarray(w_hh0[d], dtype=f32).T.copy()   # [H, 4H]
        whh0T[:, gsl] *= 2.0
        whh1T = np.asarray(w_hh1[d], dtype=f32).T.copy()
        whh1T[:, gsl] *= 2.0

        wih0T = np.concatenate([
            np.asarray(w_ih0[d], dtype=f32).T,
            (np.asarray(b_ih0[d]) + np.asarray(b_hh0[d]))
            .astype(f32)[None, :]], axis=0)                # [13, 4H]
        wih0T[:, gsl] *= 2.0

        wih1e = np.zeros((3 * H, H4), dtype=f32)
        w1 = np.asarray(w_ih1[d], dtype=f32)               # [4H, 2H]
        wih1e[0:H] = w1[:, d * H:(d + 1) * H].T
        if d == 1:
            wih1e[H:2 * H] = w1[:, 0:H].T                  # fwd-slot features
        else:
            wih1e[2 * H:3 * H] = w1[:, H:2 * H].T          # bwd-slot features
        wih1e[:, gsl] *= 2.0

        b1row = (np.asarray(b_ih1[d]) + np.asarray(b_hh1[d])).astype(f32)
        b1row = b1row[None, :].copy()
        b1row[:, gsl] *= 2.0

        fcb_t = (np.tile(np.asarray(fc_b, dtype=f32), (BSH, 1))
                 if d == 0 else np.zeros((BSH, NCLS), dtype=f32))
        # device gate order is (f, g, i, o); x16 so the fp8(e4m3)
        # recurrent weights sit in the normal range (sigma applies 1/16)
        whh0T = whh0T[:, GPERM] * 16.0
        whh1T = whh1T[:, GPERM] * 16.0
        wih0T = wih0T[:, GPERM] * 16.0
        wih1e = wih1e[:, GPERM] * 16.0
        b1row = b1row[:, GPERM] * 16.0
        in_maps.append({
            "xT": np.ascontiguousarray(xT),
            "whh0T": np.ascontiguousarray(whh0T),
            "wih0T": np.ascontiguousarray(wih0T),
            "whh1T": np.ascontiguousarray(whh1T),
            "wih1Te": wih1e,
            "b1": b1row,
            "ident": np.eye(128, dtype=f32),
            "fcwT": np.ascontiguousarray(
                np.asarray(fc_w, dtype=f32)[:, d * H:(d + 1) * H].T),
            "fcb": fcb_t,
            "out": np.zeros((BSH, NCLS), dtype=f32),
        })
    return in_maps


_NC_CACHE = {}


def kernel(x, w_ih0, w_hh0, b_ih0, b_hh0, w_ih1, w_hh1, b_ih1, b_hh1,
           fc_w, fc_b, trace=False):
    if T not in _NC_CACHE:
        _NC_CACHE[T] = build_nc(T)
    nc = _NC_CACHE[T]
    in_maps = make_in_maps(x, w_ih0, w_hh0, b_ih0, b_hh0, w_ih1, w_hh1,
                           b_ih1, b_hh1, fc_w, fc_b)
    res = run_bass_kernel_spmd(nc, in_maps, list(range(NCORES)), trace=trace)
    out = np.zeros((B, NCLS), dtype=np.float32)
    for q in range(NCORES // 2):
        out[BSH * q:BSH * q + BSH] = (res.results[2 * q]["out"]
                                      + res.results[2 * q + 1]["out"])
    kernel.last_result = res
    return out
